# revision 1
# baseline (speedup 1.0000x reference)
"""Trainium2 Bass kernel for a custom LSTM cell with LayerNorms.

Data-parallel across 8 NeuronCores: batch B=8192 is split into 8 shards of
1024 rows; weights are replicated. On-chip, activations are kept in
feature-major ("transposed") layout [feature, batch] so that:
  - gate matmuls take W.T chunks (PE-transposed, cast bf16) as the
    stationary operand and activation chunks (bf16) as the moving operand,
  - per-feature LayerNorm affine + nonlinearity fuse into single ScalarE
    activation ops (per-partition scale/bias),
  - per-batch LN statistics are ones-vector matmuls accumulating across
    feature chunks into one PSUM bank; mean/rstd rows are broadcast across
    partitions with gpsimd.partition_broadcast.
Gate activations are spilled to DRAM scratch and restreamed for the state
update to stay inside SBUF.
"""

import sys
from contextlib import ExitStack

import numpy as np

sys.path.insert(0, "/opt/trn_rl_repo")

import concourse.bass as bass
import concourse.tile as tile
from concourse import bacc, mybir
from concourse.bass_utils import run_bass_kernel_spmd
from concourse.masks import make_identity

F32 = mybir.dt.float32
BF16 = mybir.dt.bfloat16
AF = mybir.ActivationFunctionType

B, CIN, H = 8192, 512, 2048
NCORES = 8
BC = B // NCORES            # 1024 batch rows per core
NBT = BC // 128             # 8 batch row-tiles
H2 = 2 * H                  # 4096
KC = H2 // 128              # 32 contraction chunks for gate matmuls
FC = H // 128               # 16 feature chunks per gate output
PC = CIN // 128             # 4 contraction chunks for the input projection
NHB = BC // 512             # 2 PSUM batch halves (N=512 each)

GATES = ("f", "i", "c2", "o")
_PHASE_LIMIT = "full"   # profiling hook: ln | gate_f | gates | cell | full
GATE_FUNC = {"f": AF.Sigmoid, "i": AF.Sigmoid, "c2": AF.Tanh, "o": AF.Sigmoid}


def _bcast_row(row_ap, parts=128):
    """Partition-broadcast view of a [1, N] DRAM AP."""
    return bass.AP(
        tensor=row_ap.tensor,
        offset=row_ap.offset,
        ap=[[0, parts]] + [list(d) for d in row_ap.ap[1:]],
    )


def build_kernel(nc):
    ins = {}

    def din(name, shape):
        ins[name] = nc.dram_tensor(name, shape, F32, kind="ExternalInput").ap()

    din("x", (BC, 1, CIN))
    din("h", (BC, H))
    din("c", (BC, H))
    din("W_proj", (H, CIN))
    din("b_proj", (H,))
    din("g_ln", (H2,))
    din("b_ln", (H2,))
    din("g_cn", (H,))
    din("b_cn", (H,))
    din("g_hn", (H,))
    din("b_hn", (H,))
    for g in GATES:
        din(f"W_{g}", (H, H2))
        din(f"b_{g}", (H,))
        din(f"g_{g}", (H,))
        din(f"beta_{g}", (H,))

    out_h = nc.dram_tensor("out_h", (BC, H), F32, kind="ExternalOutput").ap()
    out_c = nc.dram_tensor("out_c", (BC, H), F32, kind="ExternalOutput").ap()

    with tile.TileContext(nc) as tc, ExitStack() as ctx:
        build_body(ctx, tc, ins, out_h, out_c)
    nc.compile()
    return nc


def build_body(ctx, tc, ins, out_h, out_c):
    nc = tc.nc
    global _PHASE_LIMIT

    singles = ctx.enter_context(tc.tile_pool(name="singles", bufs=1))
    stage = ctx.enter_context(tc.tile_pool(name="stage", bufs=4))
    wt_pool = ctx.enter_context(tc.tile_pool(name="wt", bufs=12))
    rows = ctx.enter_context(tc.tile_pool(name="rows", bufs=1))
    bcasts = ctx.enter_context(tc.tile_pool(name="bcasts", bufs=2))
    scratch = ctx.enter_context(tc.tile_pool(name="scratch", bufs=2))
    sq_pool = ctx.enter_context(tc.tile_pool(name="sq", bufs=2))
    tpsum = ctx.enter_context(tc.tile_pool(name="tpsum", bufs=1, space="PSUM"))
    mm_psum = ctx.enter_context(tc.tile_pool(name="mmpsum", bufs=5, space="PSUM"))
    st_psum = ctx.enter_context(tc.tile_pool(name="stpsum", bufs=2, space="PSUM"))
    dram = ctx.enter_context(tc.tile_pool(name="dram", bufs=1, space="DRAM"))

    ident = singles.tile([128, 128], F32)
    make_identity(nc, ident)
    ones_bf = singles.tile([128, 1], BF16)
    nc.vector.memset(ones_bf, 1.0)
    ones_f32 = singles.tile([128, 1], F32)
    nc.vector.memset(ones_f32, 1.0)
    eps_row = singles.tile([1, 1], F32)
    nc.vector.memset(eps_row, 1e-5)

    # Per-feature constants in chunk-column layout [128, n_chunks]:
    # element (p, c) = v[c*128 + p].
    def load_cols(name, n_chunks):
        t = singles.tile([128, n_chunks], F32, name=f"cols_{name}")
        nc.sync.dma_start(out=t, in_=ins[name].rearrange("(c p) -> p c", p=128))
        return t

    g_ln = load_cols("g_ln", KC)
    b_ln = load_cols("b_ln", KC)
    g_cn = load_cols("g_cn", FC)
    b_cn = load_cols("b_cn", FC)
    g_hn = load_cols("g_hn", FC)
    b_hn = load_cols("b_hn", FC)
    b_proj = load_cols("b_proj", FC)
    gate_g = {g: load_cols(f"g_{g}", FC) for g in GATES}
    gate_beta = {g: load_cols(f"beta_{g}", FC) for g in GATES}
    gate_b = {g: load_cols(f"b_{g}", FC) for g in GATES}

    def transpose_chunk(src_ap, dst_ap):
        """PE-transpose a [128,128] fp32 SBUF block into dst (casts via copy)."""
        pt = tpsum.tile([128, 128], F32, tag="tp")
        nc.tensor.transpose(pt, src_ap, ident)
        nc.vector.tensor_copy(out=dst_ap, in_=pt)

    # The four stats accumulation chains (sum z / sum z^2 per batch half)
    # share one PSUM bank at quadrant partitions 0/32/64/96 (walrus only
    # accepts matmul outputs at 32-aligned base partitions).
    ROFF = (0, 32, 64, 96)

    def stats_mm(stats, chunk, first, last, ones):
        for hb in range(NHB):
            zs = chunk[:, bass.ts(hb, 512)]
            sq = sq_pool.tile([128, 512], chunk.dtype, tag="sq")
            nc.scalar.square(sq, zs)
            r0, r1 = ROFF[2 * hb], ROFF[2 * hb + 1]
            nc.tensor.matmul(stats[r0 : r0 + 1, :], ones, zs,
                             start=first, stop=last, tile_position=(0, r0))
            nc.tensor.matmul(stats[r1 : r1 + 1, :], ones, sq,
                             start=first, stop=last, tile_position=(0, r1))

    def stats_to_bcast(stats, d):
        """[4,512] stats PSUM -> broadcast tiles (a_bc, c_bc) [128, BC] such
        that z_norm = z * a_bc + c_bc."""
        m = rows.tile([1, BC], F32, tag="mrow")
        v = rows.tile([1, BC], F32, tag="vrow")
        for hb in range(NHB):
            s = bass.ts(hb, 512)
            r0, r1 = ROFF[2 * hb], ROFF[2 * hb + 1]
            nc.vector.tensor_scalar_mul(m[:, s], stats[r0 : r0 + 1, :], 1.0 / d)
            nc.vector.tensor_scalar_mul(v[:, s], stats[r1 : r1 + 1, :], 1.0 / d)
        msq = rows.tile([1, BC], F32, tag="msq")
        nc.vector.tensor_mul(msq, m, m)
        nc.vector.tensor_sub(v, v, msq)          # v = var
        nc.scalar.activation(out=v, in_=v, func=AF.Sqrt, bias=eps_row, scale=1.0)
        nc.vector.reciprocal(out=v, in_=v)       # v = rstd
        nc.vector.tensor_mul(msq, m, v)
        nc.vector.tensor_scalar_mul(msq, msq, -1.0)  # msq = -m*rstd
        # Broadcast across partitions via a DRAM roundtrip (stride-0
        # partition APs are only legal with a DRAM source).
        a_bc = bcasts.tile([128, BC], F32, tag="abc")
        c_bc = bcasts.tile([128, BC], F32, tag="cbc")
        for row, bc in ((v, a_bc), (msq, c_bc)):
            drow = dram.tile([1, BC], F32, name="drow", tag="drow", bufs=4)
            nc.sync.dma_start(out=drow, in_=row)
            nc.sync.dma_start(out=bc, in_=_bcast_row(drow))
        return a_bc, c_bc

    def apply_ln(z_chunk, a_bc, c_bc, g_cols, b_cols, fc, func, dst):
        """dst = func((z*a_bc + c_bc) * g[:,fc] + b[:,fc])"""
        t = scratch.tile([128, BC], F32, tag="apply")
        nc.vector.tensor_mul(t, z_chunk, a_bc)
        nc.vector.tensor_add(t, t, c_bc)
        nc.scalar.activation(out=dst, in_=t, func=func,
                             scale=g_cols[:, fc : fc + 1],
                             bias=b_cols[:, fc : fc + 1])

    # ---- Phase 0: cast all weights to bf16 in DRAM scratch ----------------
    # Enables XBAR DMA-transposed weight loads (2-byte dtypes only), which
    # replaces per-chunk PE transposes + DVE copybacks entirely.
    wbf = {"proj": dram.tile([H, CIN], BF16, name="wbf_proj")}
    for g in GATES:
        wbf[g] = dram.tile([H, H2], BF16, name=f"wbf_{g}")

    def cast_weight(dst, src, cols):
        for fc in range(FC):
            for q in range(cols // 1024):
                ws = stage.tile([128, 1024], F32, tag="cast1024", name="cws", bufs=2)
                nc.gpsimd.dma_start(
                    out=ws, in_=src[bass.ts(fc, 128), bass.ts(q, 1024)])
                wb = stage.tile([128, 1024], BF16, tag="wbf", name="cwb", bufs=3)
                nc.gpsimd.tensor_copy(out=wb, in_=ws)
                nc.gpsimd.dma_start(
                    out=dst[bass.ts(fc, 128), bass.ts(q, 1024)], in_=wb)

    def cast_weight_512(dst, src):
        for fc in range(FC):
            ws = stage.tile([128, 512], F32, tag="cast512", name="cws5", bufs=2)
            nc.gpsimd.dma_start(out=ws, in_=src[bass.ts(fc, 128), :])
            wb = stage.tile([128, 512], BF16, tag="wbf5", name="cwb5", bufs=2)
            nc.gpsimd.tensor_copy(out=wb, in_=ws)
            nc.gpsimd.dma_start(out=dst[bass.ts(fc, 128), :], in_=wb)

    cast_weight_512(wbf["proj"], ins["W_proj"])
    for g in GATES:
        cast_weight(wbf[g], ins[f"W_{g}"], H2)

    # ---- Phase 1: load + transpose x and h --------------------------------
    # zg is allocated below comb on the pool stack: comb releases after the
    # gate matmuls while zg (holding gate o's activations in place) lives
    # through the state phase.
    zg_pool = tc.alloc_tile_pool(name="zg", bufs=1)
    comb_pool = tc.alloc_tile_pool(name="comb", bufs=1)
    xT_pool = tc.alloc_tile_pool(name="xTp", bufs=1)

    comb = [comb_pool.tile([128, BC], BF16, name=f"comb{c}", tag=f"comb{c}")
            for c in range(KC)]
    xT = [xT_pool.tile([128, BC], BF16, name=f"xT{k}", tag=f"xT{k}")
          for k in range(PC)]

    x2d = ins["x"].rearrange("b one k -> (b one) k")
    for bt in range(NBT):
        xs = stage.tile([128, 512], F32, tag="stg512")
        nc.scalar.dma_start(out=xs, in_=x2d[bass.ts(bt, 128), :])
        for j in range(PC):
            transpose_chunk(xs[:, bass.ts(j, 128)], xT[j][:, bass.ts(bt, 128)])
        for half in range(2):
            hs = stage.tile([128, 1024], F32, tag="stg1024", bufs=2)
            nc.scalar.dma_start(
                out=hs, in_=ins["h"][bass.ts(bt, 128), bass.ts(half, 1024)])
            for j in range(FC // 2):
                fc = half * (FC // 2) + j
                transpose_chunk(hs[:, bass.ts(j, 128)],
                                comb[FC + fc][:, bass.ts(bt, 128)])

    # ---- Phase 2: input projection xp^T = W_proj @ x^T + b_proj -----------
    # Feature chunks are processed in pairs: one XBAR-transposed weight load
    # [128k, 256f] feeds two PSUM accumulation chains (4 banks with NHB=2).
    comb_stats = st_psum.tile([128, 512], F32, tag="stats")

    def mm_block(dst_chunks, wsrc, xsrc, nk, bias_cols):
        """dst_chunks[f][128, BC] (bf16) = wsrc.T-chunks @ xsrc + bias.
        Feature chunks iterate in pairs. LN stats matmuls are deferred by
        the caller so the PE stream here is pure back-to-back matmuls."""
        nfc = len(dst_chunks)
        for fg in range(nfc // 2):
            zp = [[mm_psum.tile([128, 512], F32, tag="zpsum", name="zp")
                   for _ in range(NHB)] for _ in range(2)]
            for k in range(nk):
                wt = wt_pool.tile([128, 256], BF16, tag="wt")
                nc.sync.dma_start_transpose(
                    wt, wsrc[bass.ts(fg, 256), bass.ts(k, 128)])
                for f in range(2):
                    for hb in range(NHB):
                        nc.tensor.matmul(
                            zp[f][hb], wt[:, bass.ts(f, 128)],
                            xsrc[k][:, bass.ts(hb, 512)],
                            start=(k == 0), stop=(k == nk - 1))
            for f in range(2):
                fc = 2 * fg + f
                for hb in range(NHB):
                    nc.vector.tensor_scalar_add(
                        out=dst_chunks[fc][:, bass.ts(hb, 512)],
                        in0=zp[f][hb], scalar1=bias_cols[:, fc : fc + 1])

    if _PHASE_LIMIT == "prep":
        xT_pool.release(); comb_pool.release(); zg_pool.release()
        return
    mm_block(comb[:FC], wbf["proj"], xT, PC, b_proj)
    for fc in range(FC):
        stats_mm(comb_stats, comb[fc], first=(fc == 0), last=False,
                 ones=ones_bf)
    for j in range(FC):
        stats_mm(comb_stats, comb[FC + j], first=False, last=(j == FC - 1),
                 ones=ones_bf)
    xT_pool.release()

    # ---- Phase 3: combined LayerNorm + tanh (in place) --------------------
    a_bc, c_bc = stats_to_bcast(comb_stats, float(H2))
    for c in range(KC):
        apply_ln(comb[c], a_bc, c_bc, g_ln, b_ln, c, AF.Tanh, comb[c])

    if _PHASE_LIMIT == "ln":
        comb_pool.release(); zg_pool.release()
        return
    # ---- Phase 4: gates z = W_g @ comb + b_g; LN; sigmoid/tanh ------------
    # f, i, c2 activations spill to DRAM and restream in phase 5; gate o's
    # activations stay resident in the zg tiles for phase 6.
    act_dram = {g: dram.tile([H, BC], BF16, name=f"act_{g}")
                for g in GATES if g != "o"}
    o_act = None
    for g in GATES:
        wg = ins[f"W_{g}"]
        stats = st_psum.tile([128, 512], F32, tag="stats")
        zg = [zg_pool.tile([128, BC], BF16, name=f"z_{g}{fc}", tag=f"zg{fc}")
              for fc in range(FC)]
        mm_block(zg, wbf[g], comb, KC, gate_b[g])
        for fc in range(FC):
            stats_mm(stats, zg[fc], first=(fc == 0), last=(fc == FC - 1),
                     ones=ones_bf)
        a_bc, c_bc = stats_to_bcast(stats, float(H))
        for fc in range(FC):
            apply_ln(zg[fc], a_bc, c_bc, gate_g[g], gate_beta[g], fc,
                     GATE_FUNC[g], zg[fc])
            if g != "o":
                nc.scalar.dma_start(out=act_dram[g][bass.ts(fc, 128), :],
                                    in_=zg[fc])
        if g == "o":
            o_act = zg
        if _PHASE_LIMIT == "gate_f":
            comb_pool.release(); zg_pool.release()
            return

    if _PHASE_LIMIT == "gates":
        comb_pool.release(); zg_pool.release()
        return
    comb_pool.release()

    # ---- Phase 5: cell update cp = f*c + i*cc; next_cell = LN_cn(cp) ------
    state = tc.alloc_tile_pool(name="state", bufs=1)
    gs_pool = tc.alloc_tile_pool(name="gstream", bufs=2)
    asm_pool = tc.alloc_tile_pool(name="asm", bufs=2)

    cp = [state.tile([128, BC], F32, name=f"cp{j}", tag=f"cpf{j}")
          for j in range(FC)]
    cn_stats = st_psum.tile([128, 512], F32, tag="stats")
    for fc in range(FC):
        cT = gs_pool.tile([128, BC], BF16, tag="cT", bufs=2)
        for bt in range(NBT):
            cs = stage.tile([128, 128], F32, tag="stg128")
            nc.scalar.dma_start(out=cs,
                              in_=ins["c"][bass.ts(bt, 128), bass.ts(fc, 128)])
            transpose_chunk(cs, cT[:, bass.ts(bt, 128)])
        fa = gs_pool.tile([128, BC], BF16, tag="fstream", bufs=2)
        ia = gs_pool.tile([128, BC], BF16, tag="istream", bufs=2)
        ca = gs_pool.tile([128, BC], BF16, tag="cstream", bufs=2)
        nc.scalar.dma_start(out=fa, in_=act_dram["f"][bass.ts(fc, 128), :])
        nc.scalar.dma_start(out=ia, in_=act_dram["i"][bass.ts(fc, 128), :])
        nc.scalar.dma_start(out=ca, in_=act_dram["c2"][bass.ts(fc, 128), :])
        t = scratch.tile([128, BC], F32, tag="apply")
        nc.vector.tensor_mul(t, fa, cT)
        nc.vector.tensor_mul(cp[fc], ia, ca)
        nc.vector.tensor_add(cp[fc], cp[fc], t)
        stats_mm(cn_stats, cp[fc], first=(fc == 0), last=(fc == FC - 1),
                 ones=ones_f32)

    a_bc, c_bc = stats_to_bcast(cn_stats, float(H))
    hn_stats = st_psum.tile([128, 512], F32, tag="stats")
    hp = []
    for fc in range(FC):
        apply_ln(cp[fc], a_bc, c_bc, g_cn, b_cn, fc, AF.Identity, cp[fc])

    # write next_cell (transpose back to batch-major), then hidden path
    for bt in range(NBT):
        for hh in range(2):
            asm = asm_pool.tile([128, H // 2], F32, tag="asm", bufs=1)
            for j in range(FC // 2):
                fc = hh * (FC // 2) + j
                transpose_chunk(cp[fc][:, bass.ts(bt, 128)],
                                asm[:, bass.ts(j, 128)])
            nc.scalar.dma_start(
                out=out_c[bass.ts(bt, 128), bass.ts(hh, H // 2)], in_=asm)

    if _PHASE_LIMIT == "cell":
        asm_pool.release(); gs_pool.release(); state.release(); zg_pool.release()
        return
    # ---- Phase 6: hidden hp = o * tanh(next_cell); LN + tanh --------------
    for fc in range(FC):
        tcell = state.tile([128, BC], BF16, tag="tcell", bufs=2)
        nc.scalar.activation(out=tcell, in_=cp[fc], func=AF.Tanh)
        hpt = state.tile([128, BC], F32, name=f"hp{fc}", tag=f"cpf{fc}")
        nc.vector.tensor_mul(hpt, o_act[fc], tcell)
        hp.append(hpt)
        stats_mm(hn_stats, hpt, first=(fc == 0), last=(fc == FC - 1),
                 ones=ones_f32)

    a_bc, c_bc = stats_to_bcast(hn_stats, float(H))
    for fc in range(FC):
        apply_ln(hp[fc], a_bc, c_bc, g_hn, b_hn, fc, AF.Tanh, hp[fc])

    for bt in range(NBT):
        for hh in range(2):
            asm = asm_pool.tile([128, H // 2], F32, tag="asm", bufs=1)
            for j in range(FC // 2):
                fc = hh * (FC // 2) + j
                transpose_chunk(hp[fc][:, bass.ts(bt, 128)],
                                asm[:, bass.ts(j, 128)])
            nc.scalar.dma_start(
                out=out_h[bass.ts(bt, 128), bass.ts(hh, H // 2)], in_=asm)

    asm_pool.release()
    gs_pool.release()
    state.release()
    zg_pool.release()


_NC_CACHE = {}


def _get_nc():
    if "nc" not in _NC_CACHE:
        nc = bacc.Bacc(
            "TRN2",
            target_bir_lowering=False,
            debug=False,
            enable_asserts=False,
            num_devices=NCORES,
        )
        _NC_CACHE["nc"] = build_kernel(nc)
    return _NC_CACHE["nc"]


def run(inputs, **kw):
    nc = _get_nc()
    full = {k: np.ascontiguousarray(np.asarray(v, dtype=np.float32))
            for k, v in inputs.items()}
    in_maps = []
    for i in range(NCORES):
        s = slice(i * BC, (i + 1) * BC)
        m = {k: (np.ascontiguousarray(v[s]) if k in ("x", "h", "c") else v)
             for k, v in full.items()}
        in_maps.append(m)
    res = run_bass_kernel_spmd(nc, in_maps, core_ids=list(range(NCORES)), **kw)
    nh = np.concatenate([r["out_h"] for r in res.results], axis=0)
    ncl = np.concatenate([r["out_c"] for r in res.results], axis=0)
    return np.stack([nh, ncl]).astype(np.float32), res


def kernel(**inputs) -> np.ndarray:
    out, _ = run(inputs)
    return out




# revision 8
# speedup vs baseline: 2.2223x; 2.2223x over previous
"""Trainium2 Bass kernel for a custom LSTM cell with LayerNorms.

Data-parallel across 8 NeuronCores: batch B=8192 split into 8 shards of 1024
rows; weights replicated and read fp32 straight from HBM (no DRAM cast
roundtrip, no XBAR transposes).

On-chip layout strategy:
  - comb = tanh(LN(concat(x@Wp^T, h))) is built batch-major, LN stats via DVE
    bn_stats (per-partition), then PE-transposed into a feature-major bf16
    tile combT that serves as the *stationary* matmul operand for all four
    gate matmuls.
  - Gate weights stream as fp32 [128,1024] row-chunks, are PE-transposed
    (fp32 transpose matmuls into PSUM, packed 4-per-psum-tile with one
    2KB bank per block) and drained to a double-buffered bf16 W^T slice,
    the *moving* operand. One stationary load (comb block) serves 2 moving
    chunks of 512 out-features.
  - Gate outputs land batch-major: z[batch_part, out_feat]. LN stats are
    per-partition (bn_stats + bn_aggr), applied as per-partition scale/bias;
    the per-feature affine (g, beta) uses Pool tensor ops against
    partition_broadcast tiles.
  - z accumulates over 4 k-chunk-groups directly in bf16 SBUF (psum fp32
    partials, validated < 1e-2 rel err); gate biases are folded into the
    last PSUM chain via a rank-1 ones x bias-row matmul.
  - The whole state update (cell/hidden LN) is batch-major: c loads and
    h/c outputs need no transposes at all.
  - W transposes for matmul-unit u+1 are interleaved into the PSUM-chain
    gaps of unit u to keep the PE stream dense (pstate ramp) and the
    transpose PSUM tile pipelined.
"""

import sys
from contextlib import ExitStack

import numpy as np

sys.path.insert(0, "/opt/trn_rl_repo")

import concourse.bass as bass
import concourse.tile as tile
from concourse import bacc, mybir
from concourse.bass_utils import run_bass_kernel_spmd
from concourse.masks import make_identity

F32 = mybir.dt.float32
BF16 = mybir.dt.bfloat16
AF = mybir.ActivationFunctionType
OP = mybir.AluOpType

B, CIN, H = 8192, 512, 2048
NCORES = 8
BC = B // NCORES            # 1024 batch rows per core
NBT = BC // 128             # 8 batch blocks
H2 = 2 * H                  # 4096
KC = H2 // 128              # 32 contraction chunks for gate matmuls
PC = CIN // 128             # 4 contraction chunks for the input projection
NCG = 2                     # out-feature column groups of 1024
NN = 2                      # 512-wide psum chunks per column group
KCG = 4                     # k chunk groups
K8 = KC // KCG              # 8 k-chunks per group

GATES = ("c2", "i", "f", "o")
GATE_FUNC = {"f": AF.Sigmoid, "i": AF.Sigmoid, "c2": AF.Tanh, "o": AF.Sigmoid}


def build_kernel(nc):
    ins = {}

    def din(name, shape):
        ins[name] = nc.dram_tensor(name, shape, F32, kind="ExternalInput").ap()

    din("x", (BC, 1, CIN))
    din("h", (BC, H))
    din("c", (BC, H))
    din("W_proj", (H, CIN))
    din("b_proj", (H,))
    din("g_ln", (H2,))
    din("b_ln", (H2,))
    din("g_cn", (H,))
    din("b_cn", (H,))
    din("g_hn", (H,))
    din("b_hn", (H,))
    for g in GATES:
        din(f"W_{g}", (H, H2))
        din(f"b_{g}", (H,))
        din(f"g_{g}", (H,))
        din(f"beta_{g}", (H,))

    out_h = nc.dram_tensor("out_h", (BC, H), F32, kind="ExternalOutput").ap()
    out_c = nc.dram_tensor("out_c", (BC, H), F32, kind="ExternalOutput").ap()

    with tile.TileContext(nc) as tc, ExitStack() as ctx:
        build_body(ctx, tc, ins, out_h, out_c)
    nc.compile()
    return nc


def build_body(ctx, tc, ins, out_h, out_c):
    nc = tc.nc

    singles = ctx.enter_context(tc.tile_pool(name="singles", bufs=1))
    small = ctx.enter_context(tc.tile_pool(name="small", bufs=2))
    mm_psum = ctx.enter_context(tc.tile_pool(name="mmpsum", bufs=2, space="PSUM"))
    tp_psum = ctx.enter_context(tc.tile_pool(name="tppsum", bufs=1, space="PSUM"))

    ident_f = singles.tile([128, 128], F32)
    make_identity(nc, ident_f)
    ident_b = singles.tile([128, 128], BF16)
    make_identity(nc, ident_b)
    ones_bf = singles.tile([1, 128], BF16)
    nc.vector.memset(ones_bf, 1.0)
    eps_col = singles.tile([128, 1], F32)
    nc.vector.memset(eps_col, 1e-5)

    # per-feature LN constants for comb, chunk-column layout [p, c] = v[c*128+p]
    def load_cols(name, n_chunks):
        t = singles.tile([128, n_chunks], F32, name=f"cols_{name}")
        nc.sync.dma_start(out=t, in_=ins[name].rearrange("(c p) -> p c", p=128))
        return t

    glncols = load_cols("g_ln", KC)
    blncols = load_cols("b_ln", KC)

    brow = {}

    def load_row_bf(key, name):
        """[1, H] bf16 copy of a 1-D fp32 DRAM vector (casting DMA)."""
        rb = singles.tile([1, H], BF16, tag="rowb", name="rowb", bufs=1)
        nc.gpsimd.dma_start(out=rb, in_=ins[name])
        brow[key] = rb

    load_row_bf("proj", "b_proj")

    # ---- stats helper (batch-major LN) ------------------------------------
    def ln_stats(src, width, tag):
        """Per-partition (rstd, -mean*rstd) of src [128, width]."""
        nq = width // 512
        st = small.tile([128, nq, 6], F32, tag=f"st_{tag}", name="st")
        for q in range(nq):
            nc.vector.bn_stats(st[:, q, :], src[:, bass.ts(q, 512)])
        mv = small.tile([128, 2], F32, tag=f"mv_{tag}", name="mv")
        nc.vector.bn_aggr(mv, st)
        s = small.tile([128, 1], F32, tag=f"s_{tag}", name="s")
        nc.scalar.activation(out=s, in_=mv[:, 1:2], func=AF.Sqrt, bias=eps_col)
        r = small.tile([128, 1], F32, tag=f"r_{tag}", name="r")
        nc.vector.reciprocal(r, s)
        nm = small.tile([128, 1], F32, tag=f"nm_{tag}", name="nm")
        nc.vector.tensor_mul(nm, mv[:, 0:1], r)
        nc.vector.tensor_scalar_mul(nm, nm, -1.0)
        return r, nm

    # ---- Phase 1: x^T, Wp^T, comb (batch-major) + combT (feature-major) ---
    combp = tc.alloc_tile_pool(name="combp", bufs=1)
    combT = combp.tile([128, KC, BC], BF16)  # comb^T [feat, b]

    p1 = tc.alloc_tile_pool(name="p1", bufs=1)
    xT = p1.tile([128, PC, BC], BF16)       # x^T  [cin, b]
    WpT = p1.tile([128, PC, H], BF16)       # Wp^T [cin, ofeat]

    x2d = ins["x"].rearrange("b one k -> (b one) k")
    for bt in range(NBT):
        xs = p1.tile([128, CIN], F32, tag="xstage", name="xs", bufs=2)
        nc.sync.dma_start(out=xs, in_=x2d[bass.ts(bt, 128), :])
        tp = tp_psum.tile([128, 4, 512], F32, tag="tp", name="tp")
        for pc in range(PC):
            nc.tensor.transpose(tp[:, pc, 0:128], xs[:, bass.ts(pc, 128)], ident_f)
        nc.scalar.activation(out=xT[:, 0:PC, bass.ts(bt, 128)],
                             in_=tp[:, :, 0:128], func=AF.Copy)
    for oc in range(H // 128):
        ws = p1.tile([128, CIN], F32, tag="xstage", name="wps", bufs=2)
        nc.sync.dma_start(out=ws, in_=ins["W_proj"][bass.ts(oc, 128), :])
        tp = tp_psum.tile([128, 4, 512], F32, tag="tp", name="tp")
        for pc in range(PC):
            nc.tensor.transpose(tp[:, pc, 0:128], ws[:, bass.ts(pc, 128)], ident_f)
        nc.vector.tensor_copy(out=WpT[:, 0:PC, bass.ts(oc, 128)],
                              in_=tp[:, :, 0:128])

    tt = {}
    for bt in range(NBT):
        hst = p1.tile([128, H], F32, tag="hstage", name="hst", bufs=2)
        nc.sync.dma_start(out=hst, in_=ins["h"][bass.ts(bt, 128), :])
        # xp = x @ Wp^T + b_proj  (batch-major, 4 psum chains of 512)
        mm = [mm_psum.tile([128, NN, 512], F32, tag="mm", name="mm")
              for _ in range(2)]
        for pc in range(PC):
            lhs = xT[:, pc, bass.ts(bt, 128)]
            for j in range(4):
                nc.tensor.matmul(mm[j // 2][:, j % 2, :], lhs,
                                 WpT[:, pc, bass.ts(j, 512)],
                                 start=(pc == 0), stop=False)
        for j in range(4):
            nc.tensor.matmul(mm[j // 2][:, j % 2, :], ones_bf,
                             brow["proj"][:, bass.ts(j, 512)],
                             start=False, stop=True)
        craw = p1.tile([128, H2], BF16, tag="craw", name="craw", bufs=3)
        for j2 in range(2):
            nc.vector.tensor_copy(out=craw[:, bass.ts(j2, 1024)], in_=mm[j2])
        nc.scalar.activation(out=craw[:, bass.ts(1, H)], in_=hst, func=AF.Copy)
        r, nm = ln_stats(craw, H2, "c")
        t = p1.tile([128, H2], BF16, tag=f"t{bt % 4}", name="t", bufs=1)
        nc.gpsimd.tensor_scalar(out=t, in0=craw, scalar1=r, scalar2=nm,
                                op0=OP.mult, op1=OP.add)
        tt[bt] = t
        if bt % 4 == 3:
            half = bt // 4
            for kc in range(KC):
                tpb = tp_psum.tile([128, 4, 1024], BF16, tag="tp", name="tpb")
                for j in range(4):
                    nc.tensor.transpose(tpb[:, j, 0:128],
                                        tt[half * 4 + j][:, bass.ts(kc, 128)],
                                        ident_b)
                dst = combT[:, kc, bass.ts(half, 512)]
                nc.scalar.activation(
                    out=dst.rearrange("p (j b) -> p j b", j=4),
                    in_=tpb[:, :, 0:128], func=AF.Tanh,
                    scale=glncols[:, kc:kc + 1], bias=blncols[:, kc:kc + 1])
    p1.release()

    # ---- Phase 2: gates ---------------------------------------------------
    zpool = tc.alloc_tile_pool(name="zpool", bufs=1)
    wtp = tc.alloc_tile_pool(name="wtp", bufs=2)
    wstage = tc.alloc_tile_pool(name="wstage", bufs=2)
    gbcp = tc.alloc_tile_pool(name="gbcp", bufs=1)
    cellp = tc.alloc_tile_pool(name="cellp", bufs=2)

    zA = {bb: zpool.tile([128, H], BF16, tag=f"zA{bb}", name=f"zA{bb}")
          for bb in range(NBT)}
    zB = {}

    units = [(g, ncg, kcg) for g in GATES for ncg in range(NCG)
             for kcg in range(KCG)]

    def stage_dma(unit, oc):
        g, ncg, kcg = unit
        ws = wstage.tile([128, 1024], F32, tag="wstage", name="ws")
        nc.sync.dma_start(
            out=ws,
            in_=ins[f"W_{g}"][ncg * 1024 + oc * 128:ncg * 1024 + (oc + 1) * 128,
                              bass.ts(kcg, 1024)])
        return ws

    wt_tiles = {}

    def get_wt(unit):
        if unit not in wt_tiles:
            wt_tiles[unit] = wtp.tile([128, K8, 1024], BF16, tag="wt", name="wt")
        return wt_tiles[unit]

    _drain_i = [0]

    def transpose_unit_oc(unit, oc, ws):
        """PE-transpose one staged fp32 row-chunk into unit's W^T slice."""
        wt = get_wt(unit)
        for q in range(2):
            tp = tp_psum.tile([128, 4, 512], F32, tag="tp", name="wtps")
            for j in range(4):
                k8 = q * 4 + j
                nc.tensor.transpose(tp[:, j, 0:128],
                                    ws[:, bass.ts(k8, 128)], ident_f)
            dst = wt[:, q * 4:(q + 1) * 4, bass.ts(oc, 128)]
            _drain_i[0] += 1
            if _drain_i[0] % 2 == 0:
                nc.scalar.activation(out=dst, in_=tp[:, :, 0:128], func=AF.Copy)
            else:
                nc.vector.tensor_copy(out=dst, in_=tp[:, :, 0:128])

    def build_affine(gname, tag_g, tag_b):
        """Partition-broadcast bf16 [128, H] tiles of g_<name>, beta/b_<name>."""
        out = []
        bname = f"beta_{gname}" if gname in GATES else f"b_{gname}"
        for tag, src in ((tag_g, f"g_{gname}"), (tag_b, bname)):
            rb = singles.tile([1, H], BF16, tag="rowb", name="affrow", bufs=1)
            nc.gpsimd.dma_start(out=rb, in_=ins[src])
            bc = gbcp.tile([128, H], BF16, tag=tag, name="bc")
            nc.gpsimd.partition_broadcast(bc, rb)
            out.append(bc)
        return out

    # prologue: stage + transpose unit 0
    pend_stages = {}
    for oc in range(8):
        pend_stages[(0, oc)] = stage_dma(units[0], oc)
    for oc in range(8):
        transpose_unit_oc(units[0], oc, pend_stages.pop((0, oc)))

    gbc, betabc = {}, {}
    cst_tiles = {}

    def state_out(src_bf, gb, bb_, dst_dram, tag):
        """LN-affine src (per-partition stats) to fp32 halves, DMA out.
        Returns the two half tiles for further use."""
        r, nm = ln_stats(src_bf, H, tag)
        halves = []
        for hh in range(2):
            cf = cellp.tile([128, H // 2], F32, tag="cell", name="cf")
            nc.vector.tensor_scalar(out=cf, in0=src_bf[:, bass.ts(hh, H // 2)],
                                    scalar1=r, scalar2=nm,
                                    op0=OP.mult, op1=OP.add)
            nc.gpsimd.tensor_mul(cf, cf, gb[0][:, bass.ts(hh, H // 2)])
            nc.gpsimd.tensor_add(cf, cf, gb[1][:, bass.ts(hh, H // 2)])
            halves.append(cf)
        return halves

    def apply_gate(g, bb):
        """LN affine + nonlinearity on Z, then state fusion for this bb."""
        Z = zA[bb] if g == "c2" else zB[bb]
        r, nm = ln_stats(Z, H, "z")
        nc.vector.tensor_scalar(out=Z, in0=Z, scalar1=r, scalar2=nm,
                                op0=OP.mult, op1=OP.add)
        nc.gpsimd.tensor_mul(Z, Z, gbc[g])
        nc.gpsimd.tensor_add(Z, Z, betabc[g])
        nc.scalar.activation(out=Z, in_=Z, func=GATE_FUNC[g])
        if g == "i":
            nc.gpsimd.tensor_mul(zA[bb], Z, zA[bb])          # v = i * cc
        elif g == "f":
            cs = cst_tiles.pop(bb)
            nc.vector.tensor_mul(Z, Z, cs)                   # f * c
            nc.gpsimd.tensor_add(zA[bb], Z, zA[bb])          # cp = f*c + v
            # cell LN -> out_c; tc = tanh(cell) into zA
            halves = state_out(zA[bb], (gbc["cn"], betabc["cn"]), bb, out_c,
                               "cell")
            for hh, cf in enumerate(halves):
                nc.scalar.dma_start(
                    out=out_c[bass.ts(bb, 128), bass.ts(hh, H // 2)], in_=cf)
                nc.scalar.activation(out=zA[bb][:, bass.ts(hh, H // 2)],
                                     in_=cf, func=AF.Tanh)
        elif g == "o":
            nc.gpsimd.tensor_mul(zA[bb], Z, zA[bb])          # hp = o * tc
            halves = state_out(zA[bb], (gbc["hn"], betabc["hn"]), bb, out_h,
                               "hid")
            for hh, cf in enumerate(halves):
                nc.scalar.activation(out=cf, in_=cf, func=AF.Tanh)
                nc.scalar.dma_start(
                    out=out_h[bass.ts(bb, 128), bass.ts(hh, H // 2)], in_=cf)

    for ui, unit in enumerate(units):
        g, ncg, kcg = unit
        nxt = units[ui + 1] if ui + 1 < len(units) else None
        if ncg == 0 and kcg == 0:
            load_row_bf(g, f"b_{g}")
            if g != "c2":
                zB = {bb: zpool.tile([128, H], BF16, tag=f"zB{bb}",
                                     name=f"zB{bb}")
                      for bb in range(NBT)}
            gbc[g], betabc[g] = build_affine(g, "gbc", "betabc")
            if g == "f":
                gbc["cn"], betabc["cn"] = build_affine("cn", "gaff", "baff")
            if g == "o":
                gbc["hn"], betabc["hn"] = build_affine("hn", "gaff", "baff")
        if nxt is not None:
            for oc in range(2):
                pend_stages[(ui + 1, oc)] = stage_dma(nxt, oc)
        wt = get_wt(unit)
        Zs = zA if g == "c2" else zB
        last = (ncg == NCG - 1 and kcg == KCG - 1)
        for bb in range(NBT):
            mm = mm_psum.tile([128, NN, 512], F32, tag="mm", name="gmm")
            for k8 in range(K8):
                kc = kcg * K8 + k8
                lhs = combT[:, kc, bass.ts(bb, 128)]
                for j in range(NN):
                    nc.tensor.matmul(mm[:, j, :], lhs, wt[:, k8, bass.ts(j, 512)],
                                     start=(k8 == 0),
                                     stop=(k8 == K8 - 1 and kcg != KCG - 1))
            if kcg == KCG - 1:
                for j in range(NN):
                    nc.tensor.matmul(mm[:, j, :], ones_bf,
                                     brow[g][:, ncg * 1024 + j * 512:
                                             ncg * 1024 + (j + 1) * 512],
                                     start=False, stop=True)
            dst = Zs[bb][:, bass.ts(ncg, 1024)].rearrange("p (j n) -> p j n",
                                                          j=NN)
            if kcg == 0:
                nc.scalar.activation(out=dst, in_=mm, func=AF.Copy)
            else:
                nc.vector.tensor_add(dst, mm, dst)
            # prefetch c (scalar queue) during the f gate for the fusion
            if g == "f" and ncg == 1 and kcg == 0:
                cs = cellp.tile([128, H], BF16, tag="cstage", name="cs", bufs=1)
                nc.gpsimd.dma_start(out=cs, in_=ins["c"][bass.ts(bb, 128), :])
                cst_tiles[bb] = cs
            # interleave next unit's W transposes + stage DMAs into the gaps
            if nxt is not None:
                transpose_unit_oc(nxt, bb, pend_stages.pop((ui + 1, bb)))
                if bb + 2 < 8:
                    pend_stages[(ui + 1, bb + 2)] = stage_dma(nxt, bb + 2)
            if last:
                apply_gate(g, bb)
        wt_tiles.pop(unit)

    cellp.release()
    gbcp.release()
    wstage.release()
    wtp.release()
    zpool.release()
    combp.release()


_NC_CACHE = {}


def _get_nc():
    if "nc" not in _NC_CACHE:
        nc = bacc.Bacc(
            "TRN2",
            target_bir_lowering=False,
            debug=False,
            enable_asserts=False,
            num_devices=NCORES,
        )
        _NC_CACHE["nc"] = build_kernel(nc)
    return _NC_CACHE["nc"]


def run(inputs, **kw):
    nc = _get_nc()
    full = {k: np.ascontiguousarray(np.asarray(v, dtype=np.float32))
            for k, v in inputs.items()}
    in_maps = []
    for i in range(NCORES):
        s = slice(i * BC, (i + 1) * BC)
        m = {k: (np.ascontiguousarray(v[s]) if k in ("x", "h", "c") else v)
             for k, v in full.items()}
        in_maps.append(m)
    res = run_bass_kernel_spmd(nc, in_maps, core_ids=list(range(NCORES)), **kw)
    nh = np.concatenate([r["out_h"] for r in res.results], axis=0)
    ncl = np.concatenate([r["out_c"] for r in res.results], axis=0)
    return np.stack([nh, ncl]).astype(np.float32), res


def kernel(**inputs) -> np.ndarray:
    out, _ = run(inputs)
    return out
